# revision 30
# baseline (speedup 1.0000x reference)
"""VQ codebook-lookup kernel for one TRN2 chip (8 NeuronCores, SPMD).

Token-parallel sharding: the flattened token axis N*H*W = 16384 is split
into 8 shards of 2048 tokens; the [4096, 512] codebook is replicated.
Each core computes its distance block, argmin, gather and the
straight-through output locally; no collectives.

Numerics: the reference computes
    d[t,k] = fl(fl(A_t + B_k) - 2*mm[t,k])     (all f32)
and takes argmin (first occurrence on ties). Because A_t ~ 512 dominates,
d is quantized to a ~6e-5 grid; faithful replication of the two rounded
adds makes the argmin robust to ~1e-6 absolute noise in mm (measured:
0/16384 flips at 1e-7). The matmul runs as three bf16 hi/lo passes
(zh@ch + zh@cl + zl@ch, f32 PSUM accumulate), whose error is ~1.3e-7 —
f32-faithful at bf16 PE speed. We compute nd = -d via exact negation
symmetry (nd = fl(negA+negB) + 2m with negA=-A, negB=-B) so that the DVE
MAX8/MAX_INDEX pair yields argmin with first-occurrence tie-break.

The reference's straight-through output ze + fl(zq - ze) equals the
gathered codebook row zq up to one f32 rounding at |ze| scale (~2.4e-7
per element, 2.2e-5 global relative error, 1000x inside the accuracy
gate), so the kernel emits zq directly.
"""

import sys

for _p in ("/opt/trn_rl_repo",):
    if _p not in sys.path:
        sys.path.insert(0, _p)

import numpy as np
import ml_dtypes

N = 4
C = 512
H = 64
W = 64
K = 4096
T = N * H * W          # 16384 tokens
NCORES = 8
TC = T // NCORES       # 2048 tokens per core
P = 128                # partition tile
NT = TC // P           # 16 token tiles per core
KT = 512               # k-tile width (one PSUM bank)
NKT = K // KT          # 8 k tiles
CC = C // P            # 4 contraction chunks

_BF16 = ml_dtypes.bfloat16


def _build_graph():
    import concourse.bass as bass
    import concourse.mybir as mybir
    from concourse import bacc
    from concourse.tile import TileContext

    f32 = mybir.dt.float32
    bf16 = mybir.dt.bfloat16
    u32 = mybir.dt.uint32
    add = mybir.AluOpType.add
    Copy = mybir.ActivationFunctionType.Copy

    nc = bacc.Bacc("TRN2", target_bir_lowering=False, debug=False,
                   num_devices=NCORES)

    zh_ext = nc.dram_tensor("zh", [C, TC], bf16, kind="ExternalInput").ap()
    zl_ext = nc.dram_tensor("zl", [C, TC], bf16, kind="ExternalInput").ap()
    c2h_ext = nc.dram_tensor("c2h", [C, K], bf16, kind="ExternalInput").ap()
    c2l_ext = nc.dram_tensor("c2l", [C, K], bf16, kind="ExternalInput").ap()
    negB_ext = nc.dram_tensor("negb1", [1, K], f32, kind="ExternalInput").ap()
    negA_ext = nc.dram_tensor("negA", [P, NT], f32, kind="ExternalInput").ap()
    cb_ext = nc.dram_tensor("cb", [K, C], f32, kind="ExternalInput").ap()
    out_ext = nc.dram_tensor("out", [TC, C], f32, kind="ExternalOutput").ap()

    with TileContext(nc) as tc:
        with (
            tc.tile_pool(name="const", bufs=1) as const_pool,
            tc.tile_pool(name="nd", bufs=2) as nd_pool,
            tc.tile_pool(name="small", bufs=4) as small_pool,
            tc.tile_pool(name="ste", bufs=3) as ste_pool,
            tc.tile_pool(name="mm_ps", bufs=8, space="PSUM") as mm_ps_pool,
        ):
            # Per-(chunk, token-tile) pieces of zh/zl so early matmul
            # groups depend on ~32KB DMAs, and per-(chunk, k-tile) pieces
            # of the codebook. Issue order = first use order.
            zh_sb = [[None] * NT for _ in range(CC)]
            zl_sb = [[None] * NT for _ in range(CC)]
            c2h_sb = [[None] * NKT for _ in range(CC)]
            c2l_sb = [[None] * NKT for _ in range(CC)]

            def load_z(j):
                ts_ = slice(j * P, (j + 1) * P)
                for cc in range(CC):
                    rows = slice(cc * P, (cc + 1) * P)
                    t = const_pool.tile([P, P], bf16, tag=f"zh{cc}j{j}")
                    nc.sync.dma_start(out=t[:], in_=zh_ext[rows, ts_])
                    zh_sb[cc][j] = t
                    t = const_pool.tile([P, P], bf16, tag=f"zl{cc}j{j}")
                    nc.sync.dma_start(out=t[:], in_=zl_ext[rows, ts_])
                    zl_sb[cc][j] = t

            negB_row = const_pool.tile([1, K], f32, tag="negBrow")
            nc.sync.dma_start(out=negB_row[:], in_=negB_ext[:, :])
            ones_sb = const_pool.tile([1, P], f32, tag="ones")
            nc.gpsimd.memset(ones_sb[:], 1.0)
            load_z(0)
            negA_sb = const_pool.tile([P, NT], f32, tag="negA")
            nc.sync.dma_start(out=negA_sb[:], in_=negA_ext[:, :])
            negB_sb = [None] * NKT
            for kt in range(NKT):
                ks = slice(kt * KT, (kt + 1) * KT)
                for cc in range(CC):
                    rows = slice(cc * P, (cc + 1) * P)
                    th = const_pool.tile([P, KT], bf16, tag=f"c2h{cc}k{kt}")
                    nc.sync.dma_start(out=th[:], in_=c2h_ext[rows, ks])
                    tl = const_pool.tile([P, KT], bf16, tag=f"c2l{cc}k{kt}")
                    nc.sync.dma_start(out=tl[:], in_=c2l_ext[rows, ks])
                    c2h_sb[cc][kt] = th
                    c2l_sb[cc][kt] = tl
                negB_sb[kt] = const_pool.tile([P, KT], f32, tag=f"negBk{kt}",
                                              name=f"negBk{kt}")
                if kt == 0:
                    load_z(1)

            for j in range(2, NT):
                load_z(j)

            for j in range(NT):
                nd = nd_pool.tile([P, K], f32, tag="nd")

                for kt in range(NKT):
                    ks = slice(kt * KT, (kt + 1) * KT)
                    if j == 0:
                        # replicate negB across partitions on-chip: a
                        # 1-contraction f32 ones-matmul is exact (fl(1*x)=x)
                        # and doubles as PE/HAM warm-up during the initial
                        # DMA-bound window.
                        ps_b = mm_ps_pool.tile([P, KT], f32, tag="mm")
                        nc.tensor.matmul(out=ps_b[:], lhsT=ones_sb[:],
                                         rhs=negB_row[:, ks])
                        nc.scalar.activation(out=negB_sb[kt][:], in_=ps_b[:],
                                             func=Copy)
                    # nd slice = t1n = fl(negA + negB)  (one rounded add,
                    # mirroring the reference's A+B broadcast add)
                    nc.vector.tensor_scalar(
                        out=nd[:, ks], in0=negB_sb[kt][:],
                        scalar1=negA_sb[:, j:j + 1], scalar2=None, op0=add,
                    )
                    ps = mm_ps_pool.tile([P, KT], f32, tag="mm")
                    for cc in range(CC):
                        nc.tensor.matmul(
                            out=ps[:], lhsT=zh_sb[cc][j][:],
                            rhs=c2h_sb[cc][kt][:],
                            start=(cc == 0), stop=False,
                        )
                        nc.tensor.matmul(
                            out=ps[:], lhsT=zh_sb[cc][j][:],
                            rhs=c2l_sb[cc][kt][:],
                            start=False, stop=False,
                        )
                        nc.tensor.matmul(
                            out=ps[:], lhsT=zl_sb[cc][j][:],
                            rhs=c2h_sb[cc][kt][:],
                            start=False, stop=(cc == CC - 1),
                        )
                    # nd = fl(t1n + 2m): the reference's second rounded add
                    nc.vector.tensor_tensor(
                        out=nd[:, ks], in0=ps[:], in1=nd[:, ks], op=add,
                    )
                    if kt == NKT // 2 - 1:
                        # argmax (= argmin of d) of the finished first half
                        # overlaps the second half's matmuls
                        HK = K // 2
                        mxa = small_pool.tile([P, 8], f32, tag="mxa")
                        ixa = small_pool.tile([P, 8], u32, tag="ixa")
                        nc.vector.max(out=mxa[:], in_=nd[:, 0:HK])
                        nc.vector.max_index(out=ixa[:], in_max=mxa[:],
                                            in_values=nd[:, 0:HK])

                # second-half reduction + merge. Merge keeps
                # first-occurrence tie-break: on val_a == val_b half a (the
                # lower indices) wins via is_ge.
                mxb = small_pool.tile([P, 8], f32, tag="mxb")
                ixb = small_pool.tile([P, 8], u32, tag="ixb")
                nc.vector.max(out=mxb[:], in_=nd[:, HK:K])
                nc.vector.max_index(out=ixb[:], in_max=mxb[:],
                                    in_values=nd[:, HK:K])
                mask = small_pool.tile([P, 1], u32, tag="mask")
                nc.vector.tensor_tensor(out=mask[:], in0=mxa[:, 0:1],
                                        in1=mxb[:, 0:1],
                                        op=mybir.AluOpType.is_ge)
                idx = small_pool.tile([P, 1], u32, tag="idx")
                nc.vector.tensor_scalar(
                    out=idx[:], in0=ixb[:, 0:1], scalar1=HK, scalar2=None,
                    op0=add)
                nc.vector.copy_predicated(out=idx[:], mask=mask[:],
                                          data=ixa[:, 0:1])

                # The reference's decoder_input = ze + fl(zq - ze) differs
                # from zq only by f32 rounding at |ze| scale (~2.4e-7
                # absolute, 2.2e-5 global rel err) — emit zq directly.
                zq = ste_pool.tile([P, C], f32, tag="zq")
                nc.gpsimd.indirect_dma_start(
                    out=zq[:], out_offset=None,
                    in_=cb_ext[:],
                    in_offset=bass.IndirectOffsetOnAxis(ap=idx[:, :], axis=0),
                )
                nc.sync.dma_start(out=out_ext[j * P:(j + 1) * P, :],
                                  in_=zq[:])

    nc.compile()
    return nc


_NC_CACHE = None


def _get_graph():
    global _NC_CACHE
    if _NC_CACHE is None:
        _NC_CACHE = _build_graph()
    return _NC_CACHE


def _prep_inputs(feature: np.ndarray, codebook_w: np.ndarray):
    feature = np.asarray(feature, dtype=np.float32)
    codebook_w = np.asarray(codebook_w, dtype=np.float32)

    cb2t = np.ascontiguousarray((2.0 * codebook_w).T)          # [C, K] f32
    c2h = cb2t.astype(_BF16)
    c2l = (cb2t - c2h.astype(np.float32)).astype(_BF16)
    negB = -np.sum(codebook_w * codebook_w, axis=1, dtype=np.float32)  # [K]
    negb1 = np.ascontiguousarray(negB.reshape(1, K))

    in_maps = []
    for i in range(NCORES):
        n = i // 2
        h0 = (i % 2) * (H // 2)
        zeT = np.ascontiguousarray(
            feature[n, :, h0:h0 + H // 2, :].reshape(C, TC))
        zh = zeT.astype(_BF16)
        zl = (zeT - zh.astype(np.float32)).astype(_BF16)
        negA = -np.sum(zeT * zeT, axis=0, dtype=np.float32)    # [TC]
        negA_tiles = np.ascontiguousarray(negA.reshape(NT, P).T)  # [P, NT]
        in_maps.append({
            "zh": zh, "zl": zl,
            "c2h": c2h, "c2l": c2l,
            "negb1": negb1, "negA": negA_tiles,
            "cb": codebook_w,
        })
    return in_maps


def kernel(feature: np.ndarray, codebook_w: np.ndarray) -> np.ndarray:
    from concourse.bass_utils import run_bass_kernel_spmd

    nc = _get_graph()
    in_maps = _prep_inputs(feature, codebook_w)
    res = run_bass_kernel_spmd(nc, in_maps, core_ids=list(range(NCORES)))
    out = np.concatenate(
        [np.asarray(res.results[i]["out"]) for i in range(NCORES)], axis=0)
    return out


# revision 33
# speedup vs baseline: 1.0029x; 1.0029x over previous
"""VQ codebook-lookup kernel for one TRN2 chip (8 NeuronCores, SPMD).

Token-parallel sharding: the flattened token axis N*H*W = 16384 is split
into 8 shards of 2048 tokens; the [4096, 512] codebook is replicated.
Each core computes its distance block, argmin, gather and the
straight-through output locally; no collectives.

Numerics: the reference computes
    d[t,k] = fl(fl(A_t + B_k) - 2*mm[t,k])     (all f32)
and takes argmin (first occurrence on ties). Because A_t ~ 512 dominates,
d is quantized to a ~6e-5 grid; faithful replication of the two rounded
adds makes the argmin robust to ~1e-6 absolute noise in mm (measured:
0/16384 flips at 1e-7). The matmul runs as three bf16 hi/lo passes
(zh@ch + zh@cl + zl@ch, f32 PSUM accumulate), whose error is ~1.3e-7 —
f32-faithful at bf16 PE speed. We compute nd = -d via exact negation
symmetry (nd = fl(negA+negB) + 2m with negA=-A, negB=-B) so that the DVE
MAX8/MAX_INDEX pair yields argmin with first-occurrence tie-break.

The reference's straight-through output ze + fl(zq - ze) equals the
gathered codebook row zq up to one f32 rounding at |ze| scale (~2.4e-7
per element, 2.2e-5 global relative error, 1000x inside the accuracy
gate), so the kernel emits zq directly.
"""

import sys

for _p in ("/opt/trn_rl_repo", "/root/.axon_site/_ro/trn_rl_repo"):
    if _p not in sys.path:
        sys.path.insert(0, _p)

import numpy as np
import ml_dtypes

N = 4
C = 512
H = 64
W = 64
K = 4096
T = N * H * W          # 16384 tokens
NCORES = 8
TC = T // NCORES       # 2048 tokens per core
P = 128                # partition tile
NT = TC // P           # 16 token tiles per core
KT = 512               # k-tile width (one PSUM bank)
NKT = K // KT          # 8 k tiles
CC = C // P            # 4 contraction chunks

_BF16 = ml_dtypes.bfloat16


def _build_graph():
    import concourse.bass as bass
    import concourse.mybir as mybir
    from concourse import bacc
    from concourse.tile import TileContext

    f32 = mybir.dt.float32
    bf16 = mybir.dt.bfloat16
    u32 = mybir.dt.uint32
    add = mybir.AluOpType.add
    Copy = mybir.ActivationFunctionType.Copy

    nc = bacc.Bacc("TRN2", target_bir_lowering=False, debug=False,
                   num_devices=NCORES)

    zh_ext = nc.dram_tensor("zh", [C, TC], bf16, kind="ExternalInput").ap()
    zl_ext = nc.dram_tensor("zl", [C, TC], bf16, kind="ExternalInput").ap()
    c2h_ext = nc.dram_tensor("c2h", [C, K], bf16, kind="ExternalInput").ap()
    c2l_ext = nc.dram_tensor("c2l", [C, K], bf16, kind="ExternalInput").ap()
    negB_ext = nc.dram_tensor("negb1", [1, K], f32, kind="ExternalInput").ap()
    negA_ext = nc.dram_tensor("negA", [P, NT], f32, kind="ExternalInput").ap()
    cb_ext = nc.dram_tensor("cb", [K, C], f32, kind="ExternalInput").ap()
    out_ext = nc.dram_tensor("out", [TC, C], f32, kind="ExternalOutput").ap()

    with TileContext(nc) as tc:
        with (
            tc.tile_pool(name="const", bufs=1) as const_pool,
            tc.tile_pool(name="nd", bufs=2) as nd_pool,
            tc.tile_pool(name="small", bufs=4) as small_pool,
            tc.tile_pool(name="ste", bufs=3) as ste_pool,
            tc.tile_pool(name="mm_ps", bufs=8, space="PSUM") as mm_ps_pool,
        ):
            # Per-(chunk, token-tile) pieces of zh/zl so early matmul
            # groups depend on ~32KB DMAs, and per-(chunk, k-tile) pieces
            # of the codebook. Issue order = first use order.
            zh_sb = [[None] * NT for _ in range(CC)]
            zl_sb = [[None] * NT for _ in range(CC)]
            c2h_sb = [[None] * NKT for _ in range(CC)]
            c2l_sb = [[None] * NKT for _ in range(CC)]

            def load_z(j):
                ts_ = slice(j * P, (j + 1) * P)
                for cc in range(CC):
                    rows = slice(cc * P, (cc + 1) * P)
                    t = const_pool.tile([P, P], bf16, tag=f"zh{cc}j{j}")
                    nc.sync.dma_start(out=t[:], in_=zh_ext[rows, ts_])
                    zh_sb[cc][j] = t
                    t = const_pool.tile([P, P], bf16, tag=f"zl{cc}j{j}")
                    nc.sync.dma_start(out=t[:], in_=zl_ext[rows, ts_])
                    zl_sb[cc][j] = t

            negB_row = const_pool.tile([1, K], f32, tag="negBrow")
            nc.sync.dma_start(out=negB_row[:], in_=negB_ext[:, :])
            ones_sb = const_pool.tile([1, P], f32, tag="ones")
            nc.gpsimd.memset(ones_sb[:], 1.0)
            load_z(0)
            negA_sb = const_pool.tile([P, NT], f32, tag="negA")
            nc.sync.dma_start(out=negA_sb[:], in_=negA_ext[:, :])
            negB_sb = [None] * NKT
            for kt in range(NKT):
                ks = slice(kt * KT, (kt + 1) * KT)
                for cc in range(CC):
                    rows = slice(cc * P, (cc + 1) * P)
                    th = const_pool.tile([P, KT], bf16, tag=f"c2h{cc}k{kt}")
                    nc.sync.dma_start(out=th[:], in_=c2h_ext[rows, ks])
                    tl = const_pool.tile([P, KT], bf16, tag=f"c2l{cc}k{kt}")
                    nc.sync.dma_start(out=tl[:], in_=c2l_ext[rows, ks])
                    c2h_sb[cc][kt] = th
                    c2l_sb[cc][kt] = tl
                negB_sb[kt] = const_pool.tile([P, KT], f32, tag=f"negBk{kt}",
                                              name=f"negBk{kt}")
                if kt == 0:
                    load_z(1)

            for j in range(2, NT):
                load_z(j)

            for j in range(NT):
                nd = nd_pool.tile([P, K], f32, tag="nd")

                for kt in range(NKT):
                    ks = slice(kt * KT, (kt + 1) * KT)
                    if j == 0:
                        # replicate negB across partitions on-chip: a
                        # 1-contraction f32 ones-matmul is exact (fl(1*x)=x)
                        # and doubles as PE/HAM warm-up during the initial
                        # DMA-bound window.
                        ps_b = mm_ps_pool.tile([P, KT], f32, tag="mm")
                        nc.tensor.matmul(out=ps_b[:], lhsT=ones_sb[:],
                                         rhs=negB_row[:, ks])
                        nc.scalar.activation(out=negB_sb[kt][:], in_=ps_b[:],
                                             func=Copy)
                    # nd slice = t1n = fl(negA + negB)  (one rounded add,
                    # mirroring the reference's A+B broadcast add)
                    nc.vector.tensor_scalar(
                        out=nd[:, ks], in0=negB_sb[kt][:],
                        scalar1=negA_sb[:, j:j + 1], scalar2=None, op0=add,
                    )
                    ps = mm_ps_pool.tile([P, KT], f32, tag="mm")
                    for cc in range(CC):
                        nc.tensor.matmul(
                            out=ps[:], lhsT=zh_sb[cc][j][:],
                            rhs=c2h_sb[cc][kt][:],
                            start=(cc == 0), stop=False,
                        )
                        nc.tensor.matmul(
                            out=ps[:], lhsT=zh_sb[cc][j][:],
                            rhs=c2l_sb[cc][kt][:],
                            start=False, stop=False,
                        )
                        nc.tensor.matmul(
                            out=ps[:], lhsT=zl_sb[cc][j][:],
                            rhs=c2h_sb[cc][kt][:],
                            start=False, stop=(cc == CC - 1),
                        )
                    # nd = fl(t1n + 2m): the reference's second rounded add
                    nc.vector.tensor_tensor(
                        out=nd[:, ks], in0=ps[:], in1=nd[:, ks], op=add,
                    )
                    if kt == NKT // 2 - 1:
                        # argmax (= argmin of d) of the finished first half
                        # overlaps the second half's matmuls
                        HK = K // 2
                        mxa = small_pool.tile([P, 8], f32, tag="mxa")
                        ixa = small_pool.tile([P, 8], u32, tag="ixa")
                        nc.vector.max(out=mxa[:], in_=nd[:, 0:HK])
                        nc.vector.max_index(out=ixa[:], in_max=mxa[:],
                                            in_values=nd[:, 0:HK])

                # second-half reduction + merge. Merge keeps
                # first-occurrence tie-break: on val_a == val_b half a (the
                # lower indices) wins via is_ge.
                mxb = small_pool.tile([P, 8], f32, tag="mxb")
                ixb = small_pool.tile([P, 8], u32, tag="ixb")
                nc.vector.max(out=mxb[:], in_=nd[:, HK:K])
                nc.vector.max_index(out=ixb[:], in_max=mxb[:],
                                    in_values=nd[:, HK:K])
                mask = small_pool.tile([P, 1], u32, tag="mask")
                nc.vector.tensor_tensor(out=mask[:], in0=mxa[:, 0:1],
                                        in1=mxb[:, 0:1],
                                        op=mybir.AluOpType.is_ge)
                idx = small_pool.tile([P, 1], u32, tag="idx")
                nc.vector.tensor_scalar(
                    out=idx[:], in0=ixb[:, 0:1], scalar1=HK, scalar2=None,
                    op0=add)
                nc.vector.copy_predicated(out=idx[:], mask=mask[:],
                                          data=ixa[:, 0:1])

                # The reference's decoder_input = ze + fl(zq - ze) differs
                # from zq only by f32 rounding at |ze| scale (~2.4e-7
                # absolute, 2.2e-5 global rel err) — emit zq directly.
                zq = ste_pool.tile([P, C], f32, tag="zq")
                nc.gpsimd.indirect_dma_start(
                    out=zq[:], out_offset=None,
                    in_=cb_ext[:],
                    in_offset=bass.IndirectOffsetOnAxis(ap=idx[:, :], axis=0),
                )
                nc.sync.dma_start(out=out_ext[j * P:(j + 1) * P, :],
                                  in_=zq[:])

    nc.compile()
    return nc


_NC_CACHE = None


def _get_graph():
    global _NC_CACHE
    if _NC_CACHE is None:
        _NC_CACHE = _build_graph()
    return _NC_CACHE


def _prep_inputs(feature: np.ndarray, codebook_w: np.ndarray):
    feature = np.asarray(feature, dtype=np.float32)
    codebook_w = np.asarray(codebook_w, dtype=np.float32)

    cb2t = np.ascontiguousarray((2.0 * codebook_w).T)          # [C, K] f32
    c2h = cb2t.astype(_BF16)
    c2l = (cb2t - c2h.astype(np.float32)).astype(_BF16)
    negB = -np.sum(codebook_w * codebook_w, axis=1, dtype=np.float32)  # [K]
    negb1 = np.ascontiguousarray(negB.reshape(1, K))

    in_maps = []
    for i in range(NCORES):
        n = i // 2
        h0 = (i % 2) * (H // 2)
        zeT = np.ascontiguousarray(
            feature[n, :, h0:h0 + H // 2, :].reshape(C, TC))
        zh = zeT.astype(_BF16)
        zl = (zeT - zh.astype(np.float32)).astype(_BF16)
        negA = -np.sum(zeT * zeT, axis=0, dtype=np.float32)    # [TC]
        negA_tiles = np.ascontiguousarray(negA.reshape(NT, P).T)  # [P, NT]
        in_maps.append({
            "zh": zh, "zl": zl,
            "c2h": c2h, "c2l": c2l,
            "negb1": negb1, "negA": negA_tiles,
            "cb": codebook_w,
        })
    return in_maps


def kernel(feature: np.ndarray, codebook_w: np.ndarray) -> np.ndarray:
    from concourse.bass_utils import run_bass_kernel_spmd

    nc = _get_graph()
    in_maps = _prep_inputs(feature, codebook_w)
    res = run_bass_kernel_spmd(nc, in_maps, core_ids=list(range(NCORES)))
    out = np.concatenate(
        [np.asarray(res.results[i]["out"]) for i in range(NCORES)], axis=0)
    return out


# revision 50
# speedup vs baseline: 1.0090x; 1.0061x over previous
"""VQ codebook-lookup kernel for one TRN2 chip (8 NeuronCores, SPMD).

Token-parallel sharding: the flattened token axis N*H*W = 16384 is split
into 8 shards of 2048 tokens; the [4096, 512] codebook is replicated.
Each core computes its distance block, argmin, gather and the
straight-through output locally; no collectives.

Numerics: the reference computes
    d[t,k] = fl(fl(A_t + B_k) - 2*mm[t,k])     (all f32)
and takes argmin (first occurrence on ties). Because A_t ~ 512 dominates,
d is quantized to a ~6e-5 grid; faithful replication of the two rounded
adds makes the argmin robust to ~1e-6 absolute noise in mm (measured:
0/16384 flips at 1e-7). The matmul runs as three bf16 hi/lo passes
(zh@ch + zh@cl + zl@ch, f32 PSUM accumulate), whose error is ~1.3e-7 —
f32-faithful at bf16 PE speed. We compute nd = -d via exact negation
symmetry (nd = fl(negA+negB) + 2m with negA=-A, negB=-B) so that the DVE
MAX8/MAX_INDEX pair yields argmin with first-occurrence tie-break.

The reference's straight-through output ze + fl(zq - ze) equals the
gathered codebook row zq up to one f32 rounding at |ze| scale (~2.4e-7
per element, 2.2e-5 global relative error, 1000x inside the accuracy
gate), so the kernel emits zq directly.
"""

import sys

for _p in ("/opt/trn_rl_repo", "/root/.axon_site/_ro/trn_rl_repo"):
    if _p not in sys.path:
        sys.path.insert(0, _p)

import numpy as np
import ml_dtypes

N = 4
C = 512
H = 64
W = 64
K = 4096
T = N * H * W          # 16384 tokens
NCORES = 8
TC = T // NCORES       # 2048 tokens per core
P = 128                # partition tile
NT = TC // P           # 16 token tiles per core
KT = 512               # k-tile width (one PSUM bank)
NKT = K // KT          # 8 k tiles
CC = C // P            # 4 contraction chunks

_BF16 = ml_dtypes.bfloat16


def _build_graph():
    import concourse.bass as bass
    import concourse.mybir as mybir
    from concourse import bacc
    from concourse.tile import TileContext

    f32 = mybir.dt.float32
    bf16 = mybir.dt.bfloat16
    u32 = mybir.dt.uint32
    add = mybir.AluOpType.add
    Copy = mybir.ActivationFunctionType.Copy

    nc = bacc.Bacc("TRN2", target_bir_lowering=False, debug=False,
                   num_devices=NCORES)

    zh_ext = nc.dram_tensor("zh", [C, TC], bf16, kind="ExternalInput").ap()
    zl_ext = nc.dram_tensor("zl", [C, TC], bf16, kind="ExternalInput").ap()
    c2h_ext = nc.dram_tensor("c2h", [C, K], bf16, kind="ExternalInput").ap()
    c2l_ext = nc.dram_tensor("c2l", [C, K], bf16, kind="ExternalInput").ap()
    negB_ext = nc.dram_tensor("negb1", [1, K], f32, kind="ExternalInput").ap()
    negA_ext = nc.dram_tensor("negA", [P, NT], f32, kind="ExternalInput").ap()
    cb_ext = nc.dram_tensor("cb", [K, C], f32, kind="ExternalInput").ap()
    out_ext = nc.dram_tensor("out", [TC, C], f32, kind="ExternalOutput").ap()

    with TileContext(nc) as tc:
        with (
            tc.tile_pool(name="const", bufs=1) as const_pool,
            tc.tile_pool(name="nd", bufs=2) as nd_pool,
            tc.tile_pool(name="small", bufs=4) as small_pool,
            tc.tile_pool(name="ste", bufs=3) as ste_pool,
            tc.tile_pool(name="mm_ps", bufs=8, space="PSUM") as mm_ps_pool,
        ):
            # Per-(chunk, token-tile) pieces of zh/zl so early matmul
            # groups depend on ~32KB DMAs, and per-(chunk, k-tile) pieces
            # of the codebook. Issue order = first use order.
            zh_sb = [[None] * NT for _ in range(CC)]
            zl_sb = [[None] * NT for _ in range(CC)]
            c2h_sb = [[None] * NKT for _ in range(CC)]
            c2l_sb = [[None] * NKT for _ in range(CC)]

            def load_z(j):
                ts_ = slice(j * P, (j + 1) * P)
                for cc in range(CC):
                    rows = slice(cc * P, (cc + 1) * P)
                    t = const_pool.tile([P, P], bf16, tag=f"zh{cc}j{j}")
                    nc.sync.dma_start(out=t[:], in_=zh_ext[rows, ts_])
                    zh_sb[cc][j] = t
                    t = const_pool.tile([P, P], bf16, tag=f"zl{cc}j{j}")
                    nc.sync.dma_start(out=t[:], in_=zl_ext[rows, ts_])
                    zl_sb[cc][j] = t

            negB_row = const_pool.tile([1, K], f32, tag="negBrow")
            nc.sync.dma_start(out=negB_row[:], in_=negB_ext[:, :])
            ones_sb = const_pool.tile([1, P], f32, tag="ones")
            nc.gpsimd.memset(ones_sb[:], 1.0)
            load_z(0)
            negA_sb = const_pool.tile([P, NT], f32, tag="negA")
            nc.sync.dma_start(out=negA_sb[:], in_=negA_ext[:, :])
            negB_sb = [None] * NKT
            for kt in range(NKT):
                ks = slice(kt * KT, (kt + 1) * KT)
                for cc in range(CC):
                    rows = slice(cc * P, (cc + 1) * P)
                    th = const_pool.tile([P, KT], bf16, tag=f"c2h{cc}k{kt}")
                    nc.sync.dma_start(out=th[:], in_=c2h_ext[rows, ks])
                    tl = const_pool.tile([P, KT], bf16, tag=f"c2l{cc}k{kt}")
                    nc.sync.dma_start(out=tl[:], in_=c2l_ext[rows, ks])
                    c2h_sb[cc][kt] = th
                    c2l_sb[cc][kt] = tl
                negB_sb[kt] = const_pool.tile([P, KT], f32, tag=f"negBk{kt}",
                                              name=f"negBk{kt}")
                if kt == 0:
                    load_z(1)

            for j in range(2, NT):
                load_z(j)

            for j in range(NT):
                nd = nd_pool.tile([P, K], f32, tag="nd")

                for kt in range(NKT):
                    ks = slice(kt * KT, (kt + 1) * KT)
                    if j == 0:
                        # replicate negB across partitions on-chip: a
                        # 1-contraction f32 ones-matmul is exact (fl(1*x)=x)
                        # and doubles as PE/HAM warm-up during the initial
                        # DMA-bound window.
                        ps_b = mm_ps_pool.tile([P, KT], f32, tag="mm")
                        nc.tensor.matmul(out=ps_b[:], lhsT=ones_sb[:],
                                         rhs=negB_row[:, ks])
                        nc.scalar.activation(out=negB_sb[kt][:], in_=ps_b[:],
                                             func=Copy)
                    # nd slice = t1n = fl(negA + negB)  (one rounded add,
                    # mirroring the reference's A+B broadcast add)
                    nc.vector.tensor_scalar(
                        out=nd[:, ks], in0=negB_sb[kt][:],
                        scalar1=negA_sb[:, j:j + 1], scalar2=None, op0=add,
                    )
                    ps = mm_ps_pool.tile([P, KT], f32, tag="mm")
                    for cc in range(CC):
                        nc.tensor.matmul(
                            out=ps[:], lhsT=zh_sb[cc][j][:],
                            rhs=c2h_sb[cc][kt][:],
                            start=(cc == 0), stop=False,
                        )
                        nc.tensor.matmul(
                            out=ps[:], lhsT=zh_sb[cc][j][:],
                            rhs=c2l_sb[cc][kt][:],
                            start=False, stop=False,
                        )
                        nc.tensor.matmul(
                            out=ps[:], lhsT=zl_sb[cc][j][:],
                            rhs=c2h_sb[cc][kt][:],
                            start=False, stop=(cc == CC - 1),
                        )
                    # nd = fl(t1n + 2m): the reference's second rounded add
                    nc.vector.tensor_tensor(
                        out=nd[:, ks], in0=ps[:], in1=nd[:, ks], op=add,
                    )
                    # argmax (= argmin of d) of finished parts overlaps the
                    # remaining matmuls. Normal tiles: one 2048-wide pass at
                    # the halfway point. Last tile: 1024-wide quarters with
                    # rolling merges, so only a quarter reduction and one
                    # tiny merge trail the final matmul. All merges keep
                    # first-occurrence tie-break: is_ge prefers the earlier
                    # (lower-index) part on equal values.
                    if j < NT - 1:
                        if kt == NKT // 2 - 1:
                            HK = K // 2
                            mxa = small_pool.tile([P, 8], f32, tag="mxa")
                            ixa = small_pool.tile([P, 8], u32, tag="ixa")
                            nc.vector.max(out=mxa[:], in_=nd[:, 0:HK])
                            nc.vector.max_index(out=ixa[:], in_max=mxa[:],
                                                in_values=nd[:, 0:HK])
                    elif kt % 2 == 1:
                        q = kt // 2
                        qs = slice(q * 2 * KT, (q + 1) * 2 * KT)
                        mq = small_pool.tile([P, 8], f32, tag=f"mq{q}",
                                             name=f"mq{q}")
                        iq = small_pool.tile([P, 8], u32, tag=f"iq{q}",
                                             name=f"iq{q}")
                        nc.vector.max(out=mq[:], in_=nd[:, qs])
                        nc.vector.max_index(out=iq[:], in_max=mq[:],
                                            in_values=nd[:, qs])
                        if q == 0:
                            bestv = small_pool.tile([P, 1], f32, tag="bestv")
                            besti = small_pool.tile([P, 1], u32, tag="besti")
                            nc.vector.tensor_copy(out=bestv[:],
                                                  in_=mq[:, 0:1])
                            nc.vector.tensor_copy(out=besti[:],
                                                  in_=iq[:, 0:1])
                        else:
                            # merged = (bestv < mq) ? iq+off : besti;
                            # strict less-than keeps the earlier (lower
                            # index) part on ties
                            goff = small_pool.tile([P, 1], u32,
                                                   tag=f"go{q}",
                                                   name=f"go{q}")
                            nc.vector.tensor_scalar(
                                out=goff[:], in0=iq[:, 0:1],
                                scalar1=q * 2 * KT, scalar2=None, op0=add)
                            lmask = small_pool.tile([P, 1], u32,
                                                    tag=f"lm{q}",
                                                    name=f"lm{q}")
                            nc.vector.tensor_tensor(
                                out=lmask[:], in0=bestv[:], in1=mq[:, 0:1],
                                op=mybir.AluOpType.is_lt)
                            nc.vector.copy_predicated(
                                out=besti[:], mask=lmask[:], data=goff[:])
                            nc.vector.tensor_tensor(
                                out=bestv[:], in0=bestv[:], in1=mq[:, 0:1],
                                op=mybir.AluOpType.max)

                if j < NT - 1:
                    # second-half reduction + merge
                    mxb = small_pool.tile([P, 8], f32, tag="mxb")
                    ixb = small_pool.tile([P, 8], u32, tag="ixb")
                    nc.vector.max(out=mxb[:], in_=nd[:, HK:K])
                    nc.vector.max_index(out=ixb[:], in_max=mxb[:],
                                        in_values=nd[:, HK:K])
                    mask = small_pool.tile([P, 1], u32, tag="mask")
                    nc.vector.tensor_tensor(out=mask[:], in0=mxa[:, 0:1],
                                            in1=mxb[:, 0:1],
                                            op=mybir.AluOpType.is_ge)
                    idx = small_pool.tile([P, 1], u32, tag="idx")
                    nc.vector.tensor_scalar(
                        out=idx[:], in0=ixb[:, 0:1], scalar1=HK, scalar2=None,
                        op0=add)
                    nc.vector.copy_predicated(out=idx[:], mask=mask[:],
                                              data=ixa[:, 0:1])
                else:
                    idx = besti

                # The reference's decoder_input = ze + fl(zq - ze) differs
                # from zq only by f32 rounding at |ze| scale (~2.4e-7
                # absolute, 2.2e-5 global rel err) — emit zq directly.
                zq = ste_pool.tile([P, C], f32, tag="zq")
                nc.gpsimd.indirect_dma_start(
                    out=zq[:], out_offset=None,
                    in_=cb_ext[:],
                    in_offset=bass.IndirectOffsetOnAxis(ap=idx[:, :], axis=0),
                )
                nc.sync.dma_start(out=out_ext[j * P:(j + 1) * P, :],
                                  in_=zq[:])

    nc.compile()
    return nc


_NC_CACHE = None


def _get_graph():
    global _NC_CACHE
    if _NC_CACHE is None:
        _NC_CACHE = _build_graph()
    return _NC_CACHE


def _prep_inputs(feature: np.ndarray, codebook_w: np.ndarray):
    feature = np.asarray(feature, dtype=np.float32)
    codebook_w = np.asarray(codebook_w, dtype=np.float32)

    cb2t = np.ascontiguousarray((2.0 * codebook_w).T)          # [C, K] f32
    c2h = cb2t.astype(_BF16)
    c2l = (cb2t - c2h.astype(np.float32)).astype(_BF16)
    negB = -np.sum(codebook_w * codebook_w, axis=1, dtype=np.float32)  # [K]
    negb1 = np.ascontiguousarray(negB.reshape(1, K))

    in_maps = []
    for i in range(NCORES):
        n = i // 2
        h0 = (i % 2) * (H // 2)
        zeT = np.ascontiguousarray(
            feature[n, :, h0:h0 + H // 2, :].reshape(C, TC))
        zh = zeT.astype(_BF16)
        zl = (zeT - zh.astype(np.float32)).astype(_BF16)
        negA = -np.sum(zeT * zeT, axis=0, dtype=np.float32)    # [TC]
        negA_tiles = np.ascontiguousarray(negA.reshape(NT, P).T)  # [P, NT]
        in_maps.append({
            "zh": zh, "zl": zl,
            "c2h": c2h, "c2l": c2l,
            "negb1": negb1, "negA": negA_tiles,
            "cb": codebook_w,
        })
    return in_maps


def kernel(feature: np.ndarray, codebook_w: np.ndarray) -> np.ndarray:
    from concourse.bass_utils import run_bass_kernel_spmd

    nc = _get_graph()
    in_maps = _prep_inputs(feature, codebook_w)
    res = run_bass_kernel_spmd(nc, in_maps, core_ids=list(range(NCORES)))
    out = np.concatenate(
        [np.asarray(res.results[i]["out"]) for i in range(NCORES)], axis=0)
    return out
